# revision 7
# baseline (speedup 1.0000x reference)
"""Min-max normalization kernel for Trainium2 (Bass/Tile), SPMD over 8 cores.

Problem: x of shape (16, 12, 32, 128, 128) f32. For each (i, j, k) slice of
shape (128, 128): out = (x - min) / (max - min + 1e-8), min/max over the slice.

Strategy: flatten to (6144, 16384) — 6144 independent slices of 16384 elements.
Pure data-parallel over 8 cores: 768 slices per core as 6 groups of 128 slices
(one slice per SBUF partition, min/max = free-dim reduction).

Per-core kernel (v3, ~1.7x faster than the f32 two-reduce version in the
production cost model):
  - Loads cast f32->bf16 during the SWDGE DMA: halves the SBUF-side DMA
    bytes and enables the DVE 4x (2-byte) perf modes. bf16 rounding of the
    data and min/max adds ~1.5e-3 relative error, well inside the 2e-2 gate.
  - min/max are computed as a running elementwise tensor_tensor max/min tree
    over the 2 KB chunks (InstTensorTensor supports the 4x_2p DVE perf mode;
    InstTensorReduce supports none, so a plain reduce is 4x slower), followed
    by an in-tile halving tree and one small [128,1024] tensor_reduce.
  - Normalize+store is split across engines to balance the four queues:
    2/4 of chunks: ACT activation (bf16 -> f32 tile) -> store on the SP HWDGE
    ring; 1/4: same but store issued from the ACT HWDGE ring; 1/4: DVE
    tensor_scalar in-place bf16 (4x mode) -> SWDGE cast store bf16->f32.
"""

import numpy as np

N_CORES = 8
P = 128              # partitions = slices per group
FREE = 16384         # 128*128 elements per slice
GROUPS = 6           # groups per core: 768 slices / 128
CHUNK = 2048         # free-dim chunk per DMA/compute op
EPS = 1e-8
FULL_SHAPE = (16, 12, 32, 128, 128)
# Store-path pattern per group ("sp"/"act" = ACT normalize -> f32 tile ->
# HWDGE store on that ring; "pool" = DVE normalize bf16 -> SWDGE cast store).
# First group leans on ACT (idle during the ramp while group-0 stats are
# pending); last group shifts work off ACT and spreads stores so the drain
# tail after DVE finishes is short.
_PAT_FIRST = ("sp", "act", "sp", "act")
_PAT_MID = ("sp", "act", "sp", "pool")
_PAT_LAST = ("pool", "sp", "pool", "act", "sp", "pool", "sp", "act")
STORE_PATTERNS = [_PAT_FIRST] + [_PAT_MID] * 4 + [_PAT_LAST]

_nc_cache = {}


def _build_nc(chunk=CHUNK, bufs=16, out_bufs=8, tree_bufs=2, tree_stop=512,
              patterns=None, repeat=1):
    import concourse.bacc as bacc
    import concourse.tile as tile
    from concourse import mybir

    if patterns is None:
        patterns = STORE_PATTERNS
    nchunk = FREE // chunk
    f32 = mybir.dt.float32
    bf16 = mybir.dt.bfloat16
    nc = bacc.Bacc(None, target_bir_lowering=False)
    x = nc.dram_tensor("x", [GROUPS, P, FREE], f32, kind="ExternalInput")
    y = nc.dram_tensor("y", [GROUPS, P, FREE], f32, kind="ExternalOutput")
    hwdge = {"sp": nc.sync, "act": nc.scalar}

    with tile.TileContext(nc) as tc:
        with tc.tile_pool(name="data", bufs=bufs) as data, \
             tc.tile_pool(name="outp", bufs=out_bufs) as outp, \
             tc.tile_pool(name="tree", bufs=tree_bufs) as tree, \
             tc.tile_pool(name="scal", bufs=3) as scal:
            for gi, g in enumerate(
                    [g for _ in range(repeat) for g in range(GROUPS)]):
                pat = patterns[min(g, len(patterns) - 1)]
                chunks = []
                smax = tree.tile([P, chunk], bf16, tag="smax")
                smin = tree.tile([P, chunk], bf16, tag="smin")
                for c in range(nchunk):
                    t = data.tile([P, chunk], bf16, tag="data")
                    # SWDGE cast load: HBM f32 -> SBUF bf16
                    nc.gpsimd.dma_start(
                        out=t[:, :], in_=x[g, :, c * chunk:(c + 1) * chunk]
                    )
                    # running max/min accumulate (4x_2p DVE mode, all-bf16)
                    if c == 1:
                        nc.vector.tensor_tensor(
                            out=smax[:, :], in0=chunks[0][:, :], in1=t[:, :],
                            op=mybir.AluOpType.max)
                        nc.vector.tensor_tensor(
                            out=smin[:, :], in0=chunks[0][:, :], in1=t[:, :],
                            op=mybir.AluOpType.min)
                    elif c > 1:
                        nc.vector.tensor_tensor(
                            out=smax[:, :], in0=smax[:, :], in1=t[:, :],
                            op=mybir.AluOpType.max)
                        nc.vector.tensor_tensor(
                            out=smin[:, :], in0=smin[:, :], in1=t[:, :],
                            op=mybir.AluOpType.min)
                    chunks.append(t)

                # in-tile halving tree down to tree_stop, one small reduce
                w = chunk
                while w > tree_stop:
                    h = w // 2
                    nc.vector.tensor_tensor(
                        out=smax[:, :h], in0=smax[:, :h], in1=smax[:, h:w],
                        op=mybir.AluOpType.max)
                    nc.vector.tensor_tensor(
                        out=smin[:, :h], in0=smin[:, :h], in1=smin[:, h:w],
                        op=mybir.AluOpType.min)
                    w = h

                rmax = scal.tile([P, 1], f32, tag="rmax")
                rmin = scal.tile([P, 1], f32, tag="rmin")
                inv = scal.tile([P, 1], f32, tag="inv")
                nbias = scal.tile([P, 1], f32, tag="nbias")
                nc.vector.tensor_reduce(
                    out=rmax[:, :], in_=smax[:, :w],
                    axis=mybir.AxisListType.X, op=mybir.AluOpType.max)
                nc.vector.tensor_reduce(
                    out=rmin[:, :], in_=smin[:, :w],
                    axis=mybir.AxisListType.X, op=mybir.AluOpType.min)
                # inv = 1 / (rmax - rmin + EPS); nbias = -rmin * inv
                nc.vector.tensor_scalar(
                    out=inv[:, :], in0=rmax[:, :],
                    scalar1=rmin[:, 0:1], scalar2=EPS,
                    op0=mybir.AluOpType.subtract, op1=mybir.AluOpType.add)
                nc.vector.reciprocal(out=inv[:, :], in_=inv[:, :])
                nc.vector.tensor_scalar(
                    out=nbias[:, :], in0=rmin[:, :],
                    scalar1=inv[:, 0:1], scalar2=-1.0,
                    op0=mybir.AluOpType.mult, op1=mybir.AluOpType.mult)

                for c, t in enumerate(chunks):
                    path = pat[c % len(pat)]
                    sl = slice(c * chunk, (c + 1) * chunk)
                    if path == "pool":
                        # (t - rmin) * inv on DVE in-place (4x), cast store
                        nc.vector.tensor_scalar(
                            out=t[:, :], in0=t[:, :],
                            scalar1=rmin[:, 0:1], scalar2=inv[:, 0:1],
                            op0=mybir.AluOpType.subtract,
                            op1=mybir.AluOpType.mult)
                        nc.gpsimd.dma_start(out=y[g, :, sl], in_=t[:, :])
                    else:
                        o = outp.tile([P, chunk], f32, tag="out")
                        nc.scalar.activation(
                            out=o[:, :], in_=t[:, :],
                            func=mybir.ActivationFunctionType.Identity,
                            bias=nbias[:, 0:1], scale=inv[:, 0:1])
                        hwdge[path].dma_start(out=y[g, :, sl], in_=o[:, :])
    nc.compile()
    return nc


def _get_nc():
    if "nc" not in _nc_cache:
        _nc_cache["nc"] = _build_nc()
    return _nc_cache["nc"]


def run(x: np.ndarray, trace: bool = False):
    """Shard, run on 8 cores, gather. Returns (out, BassKernelResults)."""
    from concourse.bass_utils import run_bass_kernel_spmd

    x = np.asarray(x, dtype=np.float32)
    assert x.shape == FULL_SHAPE, x.shape
    xs = x.reshape(N_CORES, GROUPS, P, FREE)
    in_maps = [{"x": np.ascontiguousarray(xs[c])} for c in range(N_CORES)]
    nc = _get_nc()
    res = run_bass_kernel_spmd(nc, in_maps, core_ids=list(range(N_CORES)),
                               trace=trace)
    out = np.stack([res.results[c]["y"] for c in range(N_CORES)])
    return out.reshape(FULL_SHAPE), res


def kernel(**inputs) -> np.ndarray:
    out, _ = run(inputs["x"], trace=False)
    return out
